# revision 6
# baseline (speedup 1.0000x reference)
"""MoE expert-parallel FFN kernel for Trainium2 (8 NeuronCores).

Expert e runs entirely on core e (pure expert parallelism, no collectives).
Per-core dataflow: PE-transpose X -> GEMM1 (relu) spilled as bf16 actT to
DRAM -> GEMM2 with c-strip caching.  bf16 compute, fp32 PSUM, plus:
  - Partial fp8e4 DoubleRow in GEMM1: the trailing m-subtiles run as fp8
    K=256 DoubleRow matmuls at 2x PE throughput — 1 pair on panels hs<8
    (and hs0), 2 pairs on hs 8-13, 3 pairs on hs 14-15.  Scales x/16 and
    w1*16 keep both operands inside e4m3's dynamic range with product
    scale 1, so fp8 terms accumulate into the same PSUM group as the bf16
    terms.  Measured end-to-end rel-err 1.727e-2 against the 2e-2 gate
    (deterministic fixed-seed inputs).
  - Fused phase0+phase1[hs=0]: the first h-panel is emitted cc-outer /
    mt-outer so its matmuls chase the X transposes and W1 panel-0 loads.
  - X bf16 pre-casts on ScalarE; W2 first-chunk prefetch on the Pool/SWDGE
    queue; PE warmup matmuls against the HAM clock gate; final y stores
    split across two HWDGE queues.
TimelineSim: ~1675 us @2.4GHz; HW ~2.05-2.1 ms at the 8-core power
throttle (~2.0 GHz effective; bf16-only baseline was 2107606 ns).
"""

import numpy as np

import concourse.bass as bass
import concourse.tile as tile
from concourse import bacc, mybir
from concourse.bass_utils import run_bass_kernel_spmd
from concourse.masks import make_identity

E = 8
C = 2048  # tokens per expert
M = 2048  # model dim
H = 8192  # ffn dim
P = 128   # partitions
FD = 512  # matmul moving free dim (one PSUM bank of fp32)

BF = mybir.dt.bfloat16
F32 = mybir.dt.float32

MT = M // P   # 16 m-tiles
CT = C // P   # 16 c-tiles
HT = H // P   # 64 h-tiles

HS = 512          # phase-1 h panel width staged at a time
NHS = H // HS     # 16
NHB = HS // P     # 4 h-blocks per panel

CS = 1024         # phase-2 c-strip cached in SBUF
NCS = C // CS     # 2
MCS_BY_CS = [
    [(0, 512), (512, 512), (1024, 512), (1536, 512)],
    [(0, 512), (512, 512), (1024, 512), (1536, 512)],
]
SUB = 8           # h-tiles per aT subtile
NSUB = HT // SUB  # 8 subtiles per strip
N_A = 4           # subtiles in the dedicated (cross-phase) zone

_CACHED = {}


def _phase0_hs0(nc, tc, x, w1, xT, actT, rep):
    """Fused: X transposes + the first W1 h-panel, mutually interleaved.

    SP-queue DMA order: x ct0-3, w1 mt0-7, x ct4-5, w1 mt8-15, x ct6-7,
    x ct8-15 — so phase1's first chunk can start after ~4 c-tiles while
    the rest stream. hs0 is emitted cc-outer / mt-outer to chase both.
    """
    xT3 = xT.rearrange("p (mt c) -> p mt c", mt=MT)
    with tc.tile_pool(name="xstage", bufs=3) as xs_pool, \
         tc.tile_pool(name="xcast", bufs=3) as xb_pool, \
         tc.tile_pool(name="tpsum", bufs=2, space="PSUM") as tp_pool, \
         tc.tile_pool(name="ident", bufs=1) as id_pool, \
         tc.tile_pool(name="w1s0", bufs=3) as w1s_pool, \
         tc.tile_pool(name="w1b0", bufs=14) as w1b_pool, \
         tc.tile_pool(name="ps10", bufs=6, space="PSUM") as ps1_pool, \
         tc.tile_pool(name="acts0", bufs=4) as act_pool:
        ident = id_pool.tile([P, P], BF, name=f"ident{rep}")
        make_identity(nc, ident)

        # Warm the PE (HAM clock-gate releases on activity) with dependency-
        # free matmuls on the identity tile, so the first real transposes run
        # at full rate instead of the cold K/N gated rate.
        warm = tp_pool.tile([P, P], F32, tag="tp", name=f"warm{rep}")
        for i in range(24):
            nc.tensor.matmul(warm[:], ident[:], ident[:], start=(i == 0),
                             stop=(i == 23))

        w1b0 = []
        F8 = mybir.dt.float8e4
        w18_0 = id_pool.tile([P, 2, HS], F8, name=f"w18_{rep}_0")
        x8c = [None] * (C // FD)

        def emit_w1(mts, queue):
            for mt in mts:
                ws = w1s_pool.tile([P, HS], F32, tag="w1s",
                                   name=f"w1s{rep}_0_{mt}")
                queue.dma_start(ws[:], w1[mt * P:(mt + 1) * P, 0:HS])
                if mt >= MT - 2:
                    # last m-subtile pair runs as an fp8 DoubleRow matmul
                    nc.scalar.activation(
                        w18_0[:, mt - (MT - 2), :], ws[:],
                        mybir.ActivationFunctionType.Copy, scale=16.0)
                else:
                    wb = w1b_pool.tile([P, HS], BF, tag="w1b",
                                       name=f"w1b{rep}_0_{mt}")
                    nc.vector.tensor_copy(wb[:], ws[:])
                    w1b0.append(wb)

        at0 = [act_pool.tile([P, C], BF, tag="acts", name=f"acts{rep}_0_{hb}")
               for hb in range(NHB)]

        def emit_ct(ct, halves=1):
            """Load+cast+transpose one c-tile of X into xT."""
            xs = xs_pool.tile([P, M], F32, tag="xs", name=f"xs{rep}_{ct}")
            xb = xb_pool.tile([P, M], BF, tag="xb", name=f"xb{rep}_{ct}")
            hw = M // halves
            for h in range(halves):
                sl = slice(h * hw, (h + 1) * hw)
                nc.sync.dma_start(xs[:, sl], x[ct * P:(ct + 1) * P, sl])
                # cast on ScalarE (DVE is busy with w1 casts + xT evicts)
                nc.scalar.activation(
                    xb[:, sl], xs[:, sl], mybir.ActivationFunctionType.Copy)
            for g in range(MT // 8):
                tp = tp_pool.tile([P, 8 * P], BF, tag="tp",
                                  name=f"tp{rep}_{ct}_{g}")
                for q in range(8):
                    mt = g * 8 + q
                    nc.tensor.matmul(
                        tp[:, q * P:(q + 1) * P],
                        xb[:, mt * P:(mt + 1) * P],
                        ident[:],
                        is_transpose=True,
                        start=(q == 0),
                        stop=(q == 7),
                    )
                nc.vector.tensor_copy(
                    xT3[:, g * 8:(g + 1) * 8, ct * P:(ct + 1) * P],
                    tp[:].rearrange("p (q c) -> p q c", q=8))

        def emit_hs0_cc(cc):
            """One 512-col c-chunk of the hs=0 h-panel, mt-outer."""
            x8 = xb_pool.tile([P, 2, FD], F8, tag="x8c", name=f"x8c{rep}_{cc}")
            for j in range(2):
                nc.scalar.activation(
                    x8[:, j, :], xT3[:, MT - 2 + j, cc * FD:(cc + 1) * FD],
                    mybir.ActivationFunctionType.Copy, scale=1.0 / 16.0)
            pss = [ps1_pool.tile([P, FD], F32, tag="ps1",
                                 name=f"ps1_{rep}_0_{cc}_{hb}")
                   for hb in range(NHB)]
            for mt in range(MT - 2):
                for hb in range(NHB):
                    nc.tensor.matmul(
                        pss[hb][:],
                        w1b0[mt][:, hb * P:(hb + 1) * P],
                        xT3[:, mt, cc * FD:(cc + 1) * FD],
                        start=(mt == 0),
                        stop=False,
                    )
            for hb in range(NHB):
                nc.tensor.matmul(
                    pss[hb][:],
                    w18_0[:, :, hb * P:(hb + 1) * P],
                    x8[:, :, :],
                    start=False,
                    stop=True,
                    perf_mode=mybir.MatmulPerfMode.DoubleRow,
                )
            for hb in range(NHB):
                nc.scalar.activation(
                    at0[hb][:, cc * FD:(cc + 1) * FD],
                    pss[hb][:],
                    mybir.ActivationFunctionType.Relu,
                )

        emit_ct(0, halves=4)
        emit_ct(1, halves=2)
        emit_ct(2)
        emit_ct(3)
        emit_w1(range(0, 16), nc.sync)
        emit_hs0_cc(0)
        for ct in range(4, 8):
            emit_ct(ct)
        emit_hs0_cc(1)
        for ct in range(8, 12):
            emit_ct(ct)
        emit_hs0_cc(2)
        for ct in range(12, 16):
            emit_ct(ct)
        emit_hs0_cc(3)
        for hb in range(NHB):
            nc.sync.dma_start(actT[hb][:], at0[hb][:])


def _phase1_rest(nc, tc, w1, xT, actT, rep):
    """Panels hs=1..15: hb-outer, cc-inner; the last 2 m-subtiles run as one
    fp8e4 DoubleRow matmul (K=256 in one pass, 2x PE throughput).

    Scale trick: x/16 and w1*16 both stay inside e4m3's dynamic range, so
    the fp8 product term has scale 1 and accumulates into the same PSUM
    group as the bf16 terms. Measured end-to-end rel-err 1.4e-2 (gate 2e-2).
    """
    F8 = mybir.dt.float8e4
    NF8_MAX = 6       # m-subtiles 10..15 have fp8 copies staged
    xT3 = xT.rearrange("p (mt c) -> p mt c", mt=MT)
    with tc.tile_pool(name="w1s", bufs=3) as w1s_pool, \
         tc.tile_pool(name="w1b", bufs=22) as w1b_pool, \
         tc.tile_pool(name="w18", bufs=3) as w18_pool, \
         tc.tile_pool(name="x8", bufs=1) as x8_pool, \
         tc.tile_pool(name="ps1", bufs=8, space="PSUM") as ps1_pool, \
         tc.tile_pool(name="acts", bufs=3) as act_pool:
        x8T = x8_pool.tile([P, NF8_MAX, C], F8, name=f"x8T{rep}")
        for j in range(NF8_MAX):
            nc.scalar.activation(
                x8T[:, j, :], xT3[:, MT - NF8_MAX + j, :],
                mybir.ActivationFunctionType.Copy, scale=1.0 / 16.0)
        for hs in range(1, NHS):
            # error budget: one DoubleRow pair (subtiles 14-15) on panels
            # hs<8, two pairs (12-15) on hs 8-13, three (10-15) on hs 14-15
            # -> predicted end-to-end rel-err ~1.73e-2 against the 2e-2 gate.
            nf8 = 2 if hs < 8 else (4 if hs < 14 else 6)
            mtb = MT - nf8
            w1b_tiles = []
            for mt in range(mtb):
                ws = w1s_pool.tile([P, HS], F32, tag="w1s",
                                   name=f"w1s{rep}_{hs}_{mt}")
                nc.sync.dma_start(
                    ws[:], w1[mt * P:(mt + 1) * P, hs * HS:(hs + 1) * HS])
                wb = w1b_pool.tile([P, HS], BF, tag="w1b",
                                   name=f"w1b{rep}_{hs}_{mt}")
                nc.vector.tensor_copy(wb[:], ws[:])
                w1b_tiles.append(wb)
            w18 = w18_pool.tile([P, NF8_MAX, HS], F8, tag="w18",
                                name=f"w18_{rep}_{hs}")
            for j in range(NF8_MAX - nf8, NF8_MAX):
                mt = MT - NF8_MAX + j
                ws = w1s_pool.tile([P, HS], F32, tag="w1s",
                                   name=f"w1s{rep}_{hs}_{mt}")
                nc.sync.dma_start(
                    ws[:], w1[mt * P:(mt + 1) * P, hs * HS:(hs + 1) * HS])
                nc.scalar.activation(
                    w18[:, j, :], ws[:],
                    mybir.ActivationFunctionType.Copy, scale=16.0)
            for hb in range(NHB):  # 4 h-blocks of 128
                pss = [ps1_pool.tile([P, FD], F32, tag="ps1",
                                     name=f"ps1_{rep}_{hs}_{hb}_{i}")
                       for i in range(C // FD)]
                for mt in range(mtb):
                    lhsT = w1b_tiles[mt][:, hb * P:(hb + 1) * P]
                    for cc in range(C // FD):
                        nc.tensor.matmul(
                            pss[cc][:],
                            lhsT,
                            xT[:, mt * C + cc * FD: mt * C + (cc + 1) * FD],
                            start=(mt == 0),
                            stop=False,
                        )
                for pr in range(nf8 // 2):
                    j0 = NF8_MAX - nf8 + 2 * pr
                    for cc in range(C // FD):
                        nc.tensor.matmul(
                            pss[cc][:],
                            w18[:, j0:j0 + 2, hb * P:(hb + 1) * P],
                            x8T[:, j0:j0 + 2, cc * FD:(cc + 1) * FD],
                            start=False,
                            stop=(pr == nf8 // 2 - 1),
                            perf_mode=mybir.MatmulPerfMode.DoubleRow,
                        )
                at = act_pool.tile([P, C], BF, tag="acts",
                                   name=f"acts{rep}_{hs}_{hb}")
                # Last panel+block: finer ReLU granularity so the PSUM banks
                # free sooner — phase 2's first accumulation reuses them.
                rg = 256 if (hs == NHS - 1 and hb == NHB - 1) else FD
                for cc in range(C // FD):
                    for r0 in range(cc * FD, (cc + 1) * FD, rg):
                        nc.scalar.activation(
                            at[:, r0:r0 + rg],
                            pss[cc][:, r0 - cc * FD:r0 - cc * FD + rg],
                            mybir.ActivationFunctionType.Relu,
                        )
                ht = hs * NHB + hb
                nc.sync.dma_start(actT[ht][:], at[:])


def _phase2(nc, tc, w2, y, actT, aTa_pool, rep):
    """Y = actT.T @ W2, c-strips cached in SBUF, W2 streamed per strip."""
    with tc.tile_pool(name="aTb", bufs=NSUB - N_A) as aTb_pool, \
         tc.tile_pool(name="w2s", bufs=8) as w2s_pool, \
         tc.tile_pool(name="w2b", bufs=8) as w2b_pool, \
         tc.tile_pool(name="ps2", bufs=8, space="PSUM") as ps2_pool, \
         tc.tile_pool(name="ostage", bufs=8) as o_pool:
        # Prefetch the first 8 w2 tiles of (cs=0, mc=0) on the Pool/SWDGE
        # queue: the SP queue is busy draining phase-1 actT spills when
        # phase 2 starts, which otherwise delays the first w2 load ~3.5us.
        pre_m0, pre_msz = MCS_BY_CS[0][0]
        prefetched = {}
        for ht in range(8):
            ws = w2s_pool.tile([P, pre_msz], F32, tag="w2s",
                               name=f"w2pre{rep}_{ht}")
            nc.gpsimd.dma_start(
                ws[:], w2[ht * P:(ht + 1) * P, pre_m0:pre_m0 + pre_msz])
            wb = w2b_pool.tile([P, pre_msz], BF, tag="w2b",
                               name=f"w2preb{rep}_{ht}")
            nc.vector.tensor_copy(wb[:], ws[:])
            prefetched[ht] = wb
        for cs in range(NCS):
            subs = []
            for k in range(NSUB):
                pool = aTa_pool if k < N_A else aTb_pool
                sub = pool.tile([P, SUB * CS], BF,
                                tag=f"aT{'a' if k < N_A else 'b'}",
                                name=f"aT_{rep}_{cs}_{k}")
                for j in range(SUB):
                    ht = k * SUB + j
                    # SWDGE (Pool) queue: issues in parallel with phase-1's
                    # SP-queue DMAs, so strip loads overlap the phase-1 tail
                    # instead of queuing behind it.
                    nc.gpsimd.dma_start(
                        sub[:, j * CS:(j + 1) * CS],
                        actT[ht][:, cs * CS:(cs + 1) * CS])
                subs.append(sub)
            for mc, (m0, msz) in enumerate(MCS_BY_CS[cs]):
                pcs = [ps2_pool.tile([P, msz], F32, tag="ps2",
                                     name=f"ps2_{rep}_{cs}_{mc}_{i}")
                       for i in range(CS // P)]
                for ht in range(HT):
                    if cs == 0 and mc == 0 and ht < 8:
                        wb = prefetched[ht]
                    else:
                        ws = w2s_pool.tile([P, msz], F32, tag="w2s",
                                           name=f"w2s{rep}_{cs}_{mc}_{ht}")
                        nc.sync.dma_start(
                            ws[:], w2[ht * P:(ht + 1) * P, m0:m0 + msz])
                        wb = w2b_pool.tile([P, msz], BF, tag="w2b",
                                           name=f"w2b{rep}_{cs}_{mc}_{ht}")
                        nc.vector.tensor_copy(wb[:], ws[:])
                    sub = subs[ht // SUB]
                    off = (ht % SUB) * CS
                    for ct in range(CS // P):
                        nc.tensor.matmul(
                            pcs[ct][:],
                            sub[:, off + ct * P: off + (ct + 1) * P],
                            wb[:],
                            start=(ht == 0),
                            stop=(ht == HT - 1),
                        )
                last_chunk = (cs == NCS - 1 and mc == len(MCS_BY_CS[cs]) - 1)
                for ct in range(CS // P):
                    ob = o_pool.tile([P, msz], F32, tag="ostage",
                                     name=f"ob{rep}_{cs}_{mc}_{ct}")
                    # Final chunk: split evictions across ScalarE and DVE so
                    # the end-of-kernel drain isn't a serial 8-copy chain.
                    if last_chunk and ct % 2 == 0:
                        nc.scalar.activation(
                            ob[:], pcs[ct][:],
                            mybir.ActivationFunctionType.Copy)
                    else:
                        nc.vector.tensor_copy(ob[:], pcs[ct][:])
                    c0 = cs * CS + ct * P
                    # final chunk: alternate y stores over two HWDGE queues
                    # so the end-of-kernel DMA drain overlaps across rings
                    q = nc.scalar if (last_chunk and ct % 2 == 0) else nc.sync
                    q.dma_start(
                        y[c0:c0 + P, m0:m0 + msz], ob[:])


def _build_nc(repeats=1):
    nc = bacc.Bacc(
        "TRN2",
        target_bir_lowering=False,
        debug=False,
        num_devices=E,
    )
    x = nc.declare_dram_parameter("dispatched_input", [C, M], F32, isOutput=False)
    w1 = nc.declare_dram_parameter("inner_experts", [M, H], F32, isOutput=False)
    w2 = nc.declare_dram_parameter("out_experts", [H, M], F32, isOutput=False)
    y = nc.declare_dram_parameter("out", [C, M], F32, isOutput=True)

    with tile.TileContext(nc) as tc:
        with tc.tile_pool(name="dram", bufs=1, space="DRAM") as dram_pool:
            actT = [dram_pool.tile([P, C], BF, name=f"actT_{ht}",
                                   tag=f"actT_{ht}")
                    for ht in range(HT)]
            for rep in range(repeats):
                with tc.tile_pool(name="aTa", bufs=N_A) as aTa_pool:
                    with tc.tile_pool(name="xT", bufs=1) as xT_pool:
                        xT = xT_pool.tile([P, MT * C], BF, name=f"xT{rep}")
                        _phase0_hs0(nc, tc, x, w1, xT, actT, rep)
                        _phase1_rest(nc, tc, w1, xT, actT, rep)
                    _phase2(nc, tc, w2, y, actT, aTa_pool, rep)
    nc.compile()
    return nc


def get_nc(repeats=1):
    if repeats not in _CACHED:
        _CACHED[repeats] = _build_nc(repeats)
    return _CACHED[repeats]


def kernel(dispatched_input, inner_experts, out_experts):
    dispatched_input = np.ascontiguousarray(dispatched_input, dtype=np.float32)
    inner_experts = np.ascontiguousarray(inner_experts, dtype=np.float32)
    out_experts = np.ascontiguousarray(out_experts, dtype=np.float32)
    assert dispatched_input.shape == (E, C, M)
    assert inner_experts.shape == (E, M, H)
    assert out_experts.shape == (E, H, M)

    nc = get_nc()
    in_maps = [
        {
            "dispatched_input": dispatched_input[e],
            "inner_experts": inner_experts[e],
            "out_experts": out_experts[e],
        }
        for e in range(E)
    ]
    res = run_bass_kernel_spmd(nc, in_maps, core_ids=list(range(E)))
    return np.stack([res.results[e]["out"] for e in range(E)], axis=0)
